# revision 12
# baseline (speedup 1.0000x reference)
"""MSE-style custom loss on 8 Trainium2 NeuronCores.

reference: d = |input - target|; conditional 0.8 scale of d[0] when
d[0] in {3,4,5,6}; return mean(d*d).

Strategy (data-parallel, memory-bound):
  - Split the 32M-element 1-D tensors into 8 contiguous shards (4M each).
  - Per core: stream [128 x F] fp32 tiles of both operands from DRAM,
    d = a - b on the vector engine, then Square activation on the scalar
    engine with accum_out -> per-partition partial sums (one column per
    tile).  2 compute ops per element, both engines well under the DMA
    roofline (~32 MiB/core @ ~358 GB/s ~ 94 us).
  - Host: sum the 8 x [128 x TILES] partials in f64, apply the d[0]
    fixup (only touches one element), divide by N.
"""

import numpy as np

N = 33554432
N_CORES = 8
SHARD = N // N_CORES          # 4194304
P = 128
# (dma_width, is_tail) per chunk.  Big 4 MiB tiles (32 KB DMA
# descriptors per partition row) for bandwidth; progressively smaller
# tail tiles so the trailing compute after the last DMA byte is short.
# Compute runs in <=SLICE-wide sub-slices so the scalar engine pipelines
# behind the vector engine and pool slots release early.
BODY = [8192, 8192, 8192]
TAIL = [2048, 2048, 2048, 1024, 512, 512]
SLICE = 2048
assert (sum(BODY) + sum(TAIL)) * P == SHARD

_cache = {}


def _get_program():
    if "nc" in _cache:
        return _cache["nc"]

    import concourse.bass as bass
    import concourse.tile as tile
    from concourse import bacc, mybir

    nc = bacc.Bacc("TRN2", target_bir_lowering=False, debug=False)
    a_d = nc.dram_tensor("input", [SHARD], mybir.dt.float32,
                         kind="ExternalInput").ap()
    b_d = nc.dram_tensor("target", [SHARD], mybir.dt.float32,
                         kind="ExternalInput").ap()
    body_cols = sum(max(1, f // SLICE) for f in BODY)
    tail_cols = sum(max(1, f // SLICE) for f in TAIL)
    n_cols = body_cols + tail_cols
    out_d = nc.dram_tensor("partial", [P, n_cols], mybir.dt.float32,
                           kind="ExternalOutput").ap()

    def chunk_ap(base, off, f):
        return base[off:off + P * f].rearrange("(p f) -> p f", p=P, f=f)

    with tile.TileContext(nc) as tc:
        with tc.tile_pool(name="a", bufs=2) as pa, \
             tc.tile_pool(name="b", bufs=2) as pb, \
             tc.tile_pool(name="at", bufs=3) as pat, \
             tc.tile_pool(name="bt", bufs=3) as pbt, \
             tc.tile_pool(name="acc", bufs=1) as pacc:
            acc = pacc.tile([P, n_cols], mybir.dt.float32)
            off = 0
            col = 0
            for f in BODY + TAIL:
                tail = f <= SLICE
                ta = (pat if tail else pa).tile([P, f], mybir.dt.float32,
                                                tag="at" if tail else "a")
                nc.sync.dma_start(ta[:], chunk_ap(a_d, off, f))
                tb = (pbt if tail else pb).tile([P, f], mybir.dt.float32,
                                                tag="bt" if tail else "b")
                nc.sync.dma_start(tb[:], chunk_ap(b_d, off, f))
                for s in range(0, f, SLICE):
                    w = min(SLICE, f - s)
                    nc.vector.tensor_sub(ta[:, s:s + w], ta[:, s:s + w],
                                         tb[:, s:s + w])
                    nc.scalar.activation(ta[:, s:s + w], ta[:, s:s + w],
                                         mybir.ActivationFunctionType.Square,
                                         accum_out=acc[:, col:col + 1])
                    col += 1
                off += P * f
            assert col == n_cols
            # Issued from the scalar engine: program-order after the last
            # Square on the same engine, so no cross-engine sem hop.
            nc.scalar.dma_start(out_d[:], acc[:])

    nc.compile()
    _cache["nc"] = nc
    return nc


def run_spmd(input, target, trace=False, **kw):
    """Run the sharded kernel; returns (partial_sums_f64, BassKernelResults)."""
    from concourse.bass_utils import run_bass_kernel_spmd

    nc = _get_program()
    a = np.ascontiguousarray(np.asarray(input, dtype=np.float32)
                             ).reshape(N_CORES, SHARD)
    b = np.ascontiguousarray(np.asarray(target, dtype=np.float32)
                             ).reshape(N_CORES, SHARD)
    in_maps = [{"input": a[c], "target": b[c]} for c in range(N_CORES)]
    br = None
    for attempt in range(3):
        try:
            br = run_bass_kernel_spmd(nc, in_maps, list(range(N_CORES)),
                                      trace=trace, **kw)
            break
        except Exception:
            # Transient NRT/device hiccups (e.g. NRT_EXEC_UNIT_UNRECOVERABLE)
            # clear on retry.
            if attempt == 2:
                raise
            import time
            time.sleep(2.0)
    total = 0.0
    for r in br.results:
        total += float(np.sum(r["partial"], dtype=np.float64))
    return total, br


def kernel(input, target):
    input = np.asarray(input)
    target = np.asarray(target)
    total, _ = run_spmd(input, target)

    # res[0] fixup, faithful to the fp32 reference semantics.
    d0 = np.float32(abs(np.float32(input.reshape(-1)[0]) -
                        np.float32(target.reshape(-1)[0])))
    if d0 in (np.float32(3.0), np.float32(4.0),
              np.float32(5.0), np.float32(6.0)):
        d0f = np.float32(d0 * np.float32(0.8))
        total += float(d0f) * float(d0f) - float(d0) * float(d0)

    return np.array(total / N, dtype=np.float32)


# revision 16
# speedup vs baseline: 1.1931x; 1.1931x over previous
"""MSE-style custom loss on 8 Trainium2 NeuronCores.

reference: d = |input - target|; conditional 0.8 scale of d[0] when
d[0] in {3,4,5,6}; return mean(d*d).

Strategy (data-parallel, memory-bound):
  - Split the 32M-element 1-D tensors into 8 contiguous shards (4M each).
  - Per core: stream [128 x F] fp32 tiles of both operands from DRAM,
    d = a - b on the vector engine, then Square activation on the scalar
    engine with accum_out -> per-partition partial sums (one column per
    compute slice).  2 compute ops per element; both engines pipeline
    well under the DMA roofline (~32 MiB/core, measured ~370-410 GB/s
    sustained with 32 KB descriptors -> ~82-90 us streaming).
  - Host: sum the 8 x [128 x n_cols] partials in f64, apply the d[0]
    fixup (only touches one element), divide by N.
"""

import numpy as np

N = 33554432
N_CORES = 8
SHARD = N // N_CORES          # 4194304
P = 128
# Chunk free-dims.  Big 4 MiB body tiles (32 KB DMA descriptors per
# partition row) for bandwidth; progressively smaller tail tiles so the
# trailing compute after the last DMA byte is short.  Compute runs in
# <=SLICE-wide sub-slices so the scalar engine pipelines behind the
# vector engine and pool slots release early.
BODY = [8192, 8192, 8192]
TAIL = [2048, 2048, 2048, 1024, 512, 512]
SLICE = 2048
assert (sum(BODY) + sum(TAIL)) * P == SHARD

_cache = {}


def _get_program():
    if "nc" in _cache:
        return _cache["nc"]

    import concourse.tile as tile
    from concourse import bacc, mybir

    nc = bacc.Bacc("TRN2", target_bir_lowering=False, debug=False)
    a_d = nc.dram_tensor("input", [SHARD], mybir.dt.float32,
                         kind="ExternalInput").ap()
    b_d = nc.dram_tensor("target", [SHARD], mybir.dt.float32,
                         kind="ExternalInput").ap()
    body_cols = sum(max(1, f // SLICE) for f in BODY)
    tail_cols = sum(max(1, f // SLICE) for f in TAIL)
    n_cols = body_cols + tail_cols
    out_d = nc.dram_tensor("partial", [P, n_cols], mybir.dt.float32,
                           kind="ExternalOutput").ap()

    def chunk_ap(base, off, f):
        return base[off:off + P * f].rearrange("(p f) -> p f", p=P, f=f)

    with tile.TileContext(nc) as tc:
        with tc.tile_pool(name="a", bufs=2) as pa, \
             tc.tile_pool(name="b", bufs=2) as pb, \
             tc.tile_pool(name="at", bufs=3) as pat, \
             tc.tile_pool(name="bt", bufs=3) as pbt, \
             tc.tile_pool(name="acc", bufs=1) as pacc:
            acc = pacc.tile([P, n_cols], mybir.dt.float32)
            off = 0
            col = 0
            for f in BODY + TAIL:
                tail = f <= SLICE
                ta = (pat if tail else pa).tile([P, f], mybir.dt.float32,
                                                tag="at" if tail else "a")
                nc.sync.dma_start(ta[:], chunk_ap(a_d, off, f))
                tb = (pbt if tail else pb).tile([P, f], mybir.dt.float32,
                                                tag="bt" if tail else "b")
                nc.sync.dma_start(tb[:], chunk_ap(b_d, off, f))
                for s in range(0, f, SLICE):
                    w = min(SLICE, f - s)
                    nc.vector.tensor_sub(ta[:, s:s + w], ta[:, s:s + w],
                                         tb[:, s:s + w])
                    nc.scalar.activation(ta[:, s:s + w], ta[:, s:s + w],
                                         mybir.ActivationFunctionType.Square,
                                         accum_out=acc[:, col:col + 1])
                    col += 1
                off += P * f
            assert col == n_cols
            # Issued from the scalar engine: program-order after the last
            # Square on the same engine, so no cross-engine sem hop.
            nc.scalar.dma_start(out_d[:], acc[:])

    nc.compile()
    _cache["nc"] = nc
    return nc


def run_spmd(input, target, trace=False, **kw):
    """Run the sharded kernel; returns (partial_sums_f64, BassKernelResults)."""
    from concourse.bass_utils import run_bass_kernel_spmd

    nc = _get_program()
    a = np.ascontiguousarray(np.asarray(input, dtype=np.float32)
                             ).reshape(N_CORES, SHARD)
    b = np.ascontiguousarray(np.asarray(target, dtype=np.float32)
                             ).reshape(N_CORES, SHARD)
    in_maps = [{"input": a[c], "target": b[c]} for c in range(N_CORES)]
    br = None
    delays = [3.0, 10.0, 20.0]
    for attempt in range(len(delays) + 1):
        try:
            br = run_bass_kernel_spmd(nc, in_maps, list(range(N_CORES)),
                                      trace=trace, **kw)
            break
        except Exception:
            # Transient NRT/device hiccups (e.g. NRT_EXEC_UNIT_UNRECOVERABLE)
            # clear on retry.
            if attempt == len(delays):
                raise
            import time
            time.sleep(delays[attempt])
    total = 0.0
    for r in br.results:
        total += float(np.sum(r["partial"], dtype=np.float64))
    return total, br


def kernel(input, target):
    input = np.asarray(input)
    target = np.asarray(target)
    total, _ = run_spmd(input, target)

    # res[0] fixup, faithful to the fp32 reference semantics.
    d0 = np.float32(abs(np.float32(input.reshape(-1)[0]) -
                        np.float32(target.reshape(-1)[0])))
    if d0 in (np.float32(3.0), np.float32(4.0),
              np.float32(5.0), np.float32(6.0)):
        d0f = np.float32(d0 * np.float32(0.8))
        total += float(d0f) * float(d0f) - float(d0) * float(d0)

    return np.array(total / N, dtype=np.float32)
